# revision 40
# baseline (speedup 1.0000x reference)
"""Deformable Conv2d (adaptive, modulated) for Trainium2 — 8-core SPMD Bass kernel.

Strategy
--------
Shard (batch, H) into 8 shards: core = b*4 + hchunk, each computes 32 output
rows of one batch element.

Per core pipeline:
 1. Fused 3x3 conv (PE, f32) per group of 1024 positions produces offsets /
    adaptive-dilation / modulation rows; results are DMA-rebased into
    partition-packed [128, 1024] tiles (group g in rows g*32..g*32+17) so the
    whole coordinate stage runs once on all 128 DVE lanes instead of 18.
 2. f32 coordinate math (DVE/ACT) reproduces the reference's floor/clamp/mask
    bilinear weights exactly; one patch index u*131+v per (n,pos) with
    u,v = clamp(floor(p)+1, 0, 130) into a host-built edge-replicated
    132x132 "2x2 patch table" (entry = 4 corner pixels x 64 ch, 512 B).
 3. dma_gather (non-transpose mode, bf16, 512B tokens) runs descriptor
    generation on 4 SWDGE queues round-robin = 4 concurrent Q7 core pairs
    (transpose-mode sprays share xbar state and cannot overlap; plain CME
    writes can). Tokens land position-major.
 4. One HWDGE xbar DMA-transpose per half-group flips to channel-major
    G2[(pp,c), b*256 + r*128 + p] — matmul-ready.
 5. Corner weights mm*gx*gy are seeded in matching interleaved order,
    partition-broadcast by doubling DMAs, applied with two contiguous bf16
    tensor_tensor multiplies in place.
 6. The 4-corner bilinear sum AND the final 3x3 (stride-3) conv collapse
    into 18 PE matmuls with K=(pp,c)=128 per (r,n) tile (f32 PSUM accum).

The gather table, weight repacks, and coordinate base planes are prepared on
the host in numpy (layout/sharding prep only — no FLOPs on tensor data other
than the bf16 cast of the table).
"""

import numpy as np
import ml_dtypes

# ---- problem constants (hardcoded per contract) ----
B, C, H, W = 2, 64, 128, 128
KS, N, DIL, PAD = 3, 9, 2, 1
Hp = H + 2 * PAD            # 130
EXT = Hp + 2                # 132 (edge-replicated ext rows/cols)
NPATCH = 131 * 131          # patch-table entries (2x2 windows of ext image)
NCORES = 8
HSH = H // 4                # 32 rows per core
NPOS = HSH * W              # 4096 positions per core
NG = 4                      # groups per core
GPOS = NPOS // NG           # 1024 positions per group
NBLK = GPOS // 128          # 8 pos-blocks of 128 per group
M_CONV = 73                 # fused conv rows: off 0:18 | ad 32:50 | m 64:73
HPOS = 512                  # positions per half-group
HTOK = HPOS * N             # 4608 patch tokens per half-group
QTOK = HTOK // 2            # 2304 tokens per queue-gather

_cache = {}


# ======================================================================
# host-side input preparation
# ======================================================================

def _prep_consts(w_p, b_p, w_m, w_ad, w_conv):
    f32 = np.float32
    # fused conv taps: wt[t, c, m], t = dy*3+dx
    wt = np.zeros((9, C, M_CONV), f32)
    rep3 = [0, 1, 2, 0, 1, 2, 0, 1, 2]
    for t in range(9):
        dy, dx = t // 3, t % 3
        wt[t, :, 0:9] = w_p[0:9, :, dy, dx].T
        wt[t, :, 9:18] = w_p[9:18, :, dy, dx].T
        wt[t, :, 32:41] = w_ad[rep3, :, dy, dx].T
        wt[t, :, 41:50] = w_ad[rep3, :, dy, dx].T
        wt[t, :, 64:73] = w_m[0:9, :, dy, dx].T
    # tap-paired layout: tile m<3 pairs taps (m,0)+(m,1) over K=128 (the
    # second operand half reads the +1-column-shifted slab copy); tile 3+m
    # holds tap (m,2) over K=64.
    wtp = np.zeros((128, 6 * M_CONV), f32)
    for dy in range(3):
        wtp[0:64, dy * M_CONV:(dy + 1) * M_CONV] = wt[dy * 3 + 0]
        wtp[64:128, dy * M_CONV:(dy + 1) * M_CONV] = wt[dy * 3 + 1]
        wtp[0:64, (3 + dy) * M_CONV:(4 + dy) * M_CONV] = wt[dy * 3 + 2]
    wt = wtp

    # packed per-partition constants: group g occupies rows g*32 .. g*32+17
    bp128 = np.zeros((128, 1), f32)
    pn128 = np.zeros((128, 1), f32)
    r = np.array([-1.0, 0.0, 1.0], f32)
    pnx = np.repeat(r, 3)
    pny = np.tile(r, 3)
    pn18 = np.concatenate([pnx, pny]).astype(f32)
    for g in range(NG):
        bp128[g * 32:g * 32 + 18, 0] = b_p
        pn128[g * 32:g * 32 + 18, 0] = pn18

    # idx matmul matrix, replicated per 32-row band: idx = 131*R1x + R1y
    lconst = np.zeros((128, 9), f32)
    for g in range(NG):
        for n in range(9):
            lconst[g * 32 + n, n] = 131.0
            lconst[g * 32 + 9 + n, n] = 1.0
    # w3: [128=(c,pp): row 2c+pp], 18*64] bf16, k-tile t=(r*9+n)
    w3 = np.zeros((128, 18 * 64), f32)
    for t in range(18):
        n = t % 9
        blk = w_conv[:, :, n // 3, n % 3].T  # [c, o]
        w3[0::2, t * 64:(t + 1) * 64] = blk
        w3[1::2, t * 64:(t + 1) * 64] = blk
    w3 = w3.astype(ml_dtypes.bfloat16)

    ident = np.eye(128, dtype=f32)
    return dict(wt=wt, bp128=bp128, pn128=pn128, lconst=lconst, w3=w3,
                ident=ident)


def _prep_table(xb):
    """xb: [C, H, W] f32 -> patch table [NPATCH, 256] bf16.

    Entry (u, v) (u, v in [0, 130], idx = u*131+v) holds the 2x2 pixel
    patch of the edge-replicated 132x132 padded image at rows (u, u+1),
    cols (v, v+1), laid out (r, pp, c)."""
    xp = np.pad(xb, ((0, 0), (PAD, PAD), (PAD, PAD)))          # [C, 130, 130]
    idx = np.clip(np.arange(EXT) - 1, 0, Hp - 1)
    ext = xp[:, idx][:, :, idx]                                # [C, 132, 132]
    win = np.lib.stride_tricks.sliding_window_view(ext, (2, 2), axis=(1, 2))
    # win: [C, 131, 131, 2, 2] -> (u, v, r, c, pp): the (c,pp) interleave
    # lets the corner-weight broadcast seed a row pair and double 2->128.
    patch = np.ascontiguousarray(win.transpose(1, 2, 3, 0, 4)).reshape(NPATCH, 256)
    return patch.astype(ml_dtypes.bfloat16)


def _prep_core_inputs(core, x, consts):
    b, hc = core // 4, core % 4
    h0 = hc * HSH
    # conv input rows h0-1 .. h0+32 (34 rows), zero padded at batch edges
    xs = np.zeros((C, HSH + 2, W), np.float32)
    lo, hi = h0 - 1, h0 + HSH + 1
    slo, shi = max(lo, 0), min(hi, H)
    xs[:, slo - lo:shi - lo, :] = x[b, :, slo:shi, :]

    # packed base coords: rows g*32+j (j<18), cols = pos within group
    basep = np.zeros((128, GPOS), np.float32)
    pos = np.arange(NPOS)
    bx = (h0 + pos // W + 1).astype(np.float32)
    by = (pos % W + 1).astype(np.float32)
    for g in range(NG):
        sl = slice(g * GPOS, (g + 1) * GPOS)
        basep[g * 32:g * 32 + 9, :] = bx[sl][None, :]
        basep[g * 32 + 9:g * 32 + 18, :] = by[sl][None, :]

    m = dict(xs=xs, basep=basep, xe=_cache[('xe', b)])
    m.update({k: consts[k] for k in ('wt', 'bp128', 'pn128', 'lconst', 'w3',
                                     'ident')})
    return m


# ======================================================================
# bass program
# ======================================================================

def _emit(nc, tc, t):
    import concourse.bass as bass
    import concourse.mybir as mybir
    from concourse.bass import AP

    dt = mybir.dt
    ALU = mybir.AluOpType
    ACTF = mybir.ActivationFunctionType
    f32, bf16, i16, i32 = dt.float32, dt.bfloat16, dt.int16, dt.int32

    XROW = HSH + 2          # 34
    XCW = W + 2             # 130 padded row width in sbuf
    any_, vec, act, pe, gp, snc = nc.any, nc.vector, nc.scalar, nc.tensor, nc.gpsimd, nc.sync

    with tc.tile_pool(name="const", bufs=1) as cpool, \
         tc.tile_pool(name="work", bufs=1) as wpool, \
         tc.tile_pool(name="gath", bufs=2) as gpool, \
         tc.tile_pool(name="gath2", bufs=2) as g2pool, \
         tc.tile_pool(name="wrbp", bufs=2) as wrbpool, \
         tc.tile_pool(name="psA", bufs=2, space="PSUM") as psA, \
         tc.tile_pool(name="psB", bufs=2, space="PSUM") as psB:
        kpool = wpool

        # ---- load constants / inputs to SBUF ----
        xsb = cpool.tile([128, XROW * XCW], f32, tag="xsb")
        gp.memset(xsb[:], 0.0)
        snc.dma_start(
            out=AP(xsb.tensor, xsb[:].offset + 1,
                   [[XROW * XCW, C], [XCW, XROW], [1, W]]),
            in_=t['xs'][:])
        snc.dma_start(
            out=AP(xsb.tensor, xsb[:].offset + 64 * (XROW * XCW),
                   [[XROW * XCW, C], [XCW, XROW], [1, W]]),
            in_=t['xs'][:])
        wtb = cpool.tile([128, 6 * M_CONV], f32, tag="wtb")
        snc.dma_start(out=wtb[:], in_=t['wt'][:])
        w3b = cpool.tile([128, 18 * 64], bf16, tag="w3b")
        snc.dma_start(out=w3b[:], in_=t['w3'][:])
        idb = cpool.tile([18, 18], f32, tag="idb")
        snc.dma_start(out=idb[:], in_=t['ident'][:18, :18])
        bpb = cpool.tile([128, 1], f32, tag="bpb")
        snc.dma_start(out=bpb[:], in_=t['bp128'][:])
        pnb = cpool.tile([128, 1], f32, tag="pnb")
        snc.dma_start(out=pnb[:], in_=t['pn128'][:])
        lcb = cpool.tile([128, 9], f32, tag="lcb")
        snc.dma_start(out=lcb[:], in_=t['lconst'][:])
        mbias = cpool.tile([128, 1], f32, tag="mbias")
        gp.memset(mbias[:], -64.5)
        bslp = cpool.tile([128, GPOS], f32, tag="bslp")
        snc.dma_start(out=bslp[:], in_=t['basep'][:])

        xe = t['xe']  # dram [NPATCH, 256] bf16

        def pt(tag):
            return kpool.tile([128, GPOS], f32, tag=tag, name=tag)

        # ================= phase A: convs -> packed tiles =================
        POFF, PSGN, PSGM = pt("poff"), pt("psgn"), pt("psgm")
        for g in range(NG):
            conv = kpool.tile([M_CONV, GPOS], f32, tag="conv")
            for k in range(2):
                pc = psA.tile([M_CONV, 512], f32, tag="pc")
                for dy in range(3):
                    rhs = AP(xsb.tensor,
                             xsb[:].offset + (g * 8 + k * 4 + dy) * XCW,
                             [[XROW * XCW, 128], [XCW, 4], [1, W]])
                    pe.matmul(pc[:], wtb[:, dy * M_CONV:(dy + 1) * M_CONV],
                              rhs, start=(dy == 0), stop=False)
                for dy in range(3):
                    rhs = AP(xsb.tensor,
                             xsb[:].offset + (g * 8 + k * 4 + dy) * XCW + 2,
                             [[XROW * XCW, C], [XCW, 4], [1, W]])
                    pe.matmul(pc[:],
                              wtb[0:64, (3 + dy) * M_CONV:(4 + dy) * M_CONV],
                              rhs, start=False, stop=(dy == 2))
                sl = slice(k * 512, (k + 1) * 512)
                act.activation(conv[0:18, sl], pc[0:18, :], ACTF.Identity,
                               bias=bpb[0:18, :], scale=1.0)
                act.activation(conv[32:50, sl], pc[32:50, :], ACTF.Sigmoid,
                               scale=-1.0)
                act.activation(conv[64:73, sl], pc[64:73, :], ACTF.Sigmoid,
                               scale=1.0)
            b0 = g * 32
            snc.dma_start(out=POFF[b0:b0 + 18, :], in_=conv[0:18, :])
            snc.dma_start(out=PSGN[b0:b0 + 18, :], in_=conv[32:50, :])
            snc.dma_start(out=PSGM[b0:b0 + 9, :], in_=conv[64:73, :])

        # ======== phase B/B': coordinates + idx prep, per 64-row half ======
        idxr_t = {}
        V = pt("v")
        I32T = kpool.tile([128, GPOS], i32, tag="i32t")
        F = pt("f")
        FRAC = pt("frac")
        QRB1 = pt("qrb1")
        R1 = pt("r1")
        MASK = pt("mask")
        GLT = kpool.tile([128, GPOS], f32, tag="poff", name="glt")
        GRB = kpool.tile([128, GPOS], f32, tag="mask", name="grb")
        ADM = kpool.tile([128, GPOS], f32, tag="v", name="adm")
        T1 = kpool.tile([128, GPOS], f32, tag="frac", name="t1")
        T2 = kpool.tile([128, GPOS], f32, tag="i32t", name="t2")
        GLTY = kpool.tile([128, GPOS], f32, tag="psgm", name="glty")
        GRBY = kpool.tile([128, GPOS], f32, tag="qrb1", name="grby")
        WCT = {}
        for tag in ("w00", "w01", "w10", "w11"):
            WCT[tag] = wpool.tile([128, GPOS], bf16, tag=tag, name=tag)
        WCL = [WCT["w00"], WCT["w01"], WCT["w10"], WCT["w11"]]

        for sg in range(1):
            s = slice(0, 128)
            # ADF = 1 + 2*(1-sigmoid), in place over PSGN
            vec.tensor_scalar(PSGN[s, :], PSGN[s, :], 2.0, 1.0, ALU.mult, ALU.add)
            ADF = PSGN
            vec.tensor_tensor(V[s, :], POFF[s, :], bslp[s, :], op=ALU.add)
            vec.scalar_tensor_tensor(V[s, :], ADF[s, :], pnb[s, :], V[s, :],
                                     op0=ALU.mult, op1=ALU.add)
            # floor(V) robust to convert rounding: g=int(V); F=g-(g>V)
            vec.tensor_copy(I32T[s, :], V[s, :])
            vec.tensor_copy(F[s, :], I32T[s, :])
            vec.tensor_tensor(FRAC[s, :], F[s, :], V[s, :], op=ALU.is_gt)
            vec.tensor_tensor(F[s, :], F[s, :], FRAC[s, :], op=ALU.subtract)
            vec.tensor_tensor(FRAC[s, :], V[s, :], F[s, :], op=ALU.subtract)
            vec.tensor_scalar(QRB1[s, :], F[s, :], 1.0, 0.0, ALU.add, ALU.max)
            vec.tensor_scalar(F[s, :], F[s, :], 0.0, Hp - 1.0, ALU.max, ALU.min)
            QLT = F
            vec.tensor_scalar(R1[s, :], QRB1[s, :], Hp + 0.0, None, ALU.min)
            vec.tensor_scalar(QRB1[s, :], QRB1[s, :], Hp - 1.0, None, ALU.min)
            QRB = QRB1
            for g in range(NG):
                b0 = g * 32
                # idxf col (bl*128 + p*8 + s) <- idx[n, pos=bl*128+s*16+p]
                idxf = kpool.tile([9, GPOS], f32, tag="idxf")
                for k in range(2):
                    pi = psB.tile([9, 512], f32, tag="pi")
                    pe.matmul(pi[:], lcb[b0:b0 + 18, :],
                              R1[b0:b0 + 18, k * 512:(k + 1) * 512],
                              start=True, stop=True, tile_position=(b0, 0))
                    act.activation(
                        AP(idxf.tensor, idxf[:].offset + k * 512,
                           [[GPOS, 9], [128, 4], [1, 8], [8, 16]]),
                        pi[:], ACTF.Copy, scale=1.0)
                # idxt[p*8+s, bl*9+n] = idx[n, pos=bl*128+s*16+p] (i16)
                idxt = kpool.tile([128, NBLK * 9], i16, tag="idxt")
                for bl in range(NBLK):
                    tp = psB.tile([128, 9], f32, tag="tp")
                    pe.transpose(tp[:], idxf[:, bl * 128:(bl + 1) * 128],
                                 idb[0:9, 0:9])
                    vec.tensor_copy(idxt[:, bl * 9:bl * 9 + 9], tp[:])

                for h in range(2):
                    idxm = gpool.tile([16, HTOK // 16], i16, tag="idxm")
                    for bl in range(4):
                        act.dma_start(
                            out=AP(idxm.tensor, idxm[:].offset + bl * 72,
                                   [[HTOK // 16, 16], [9, 8], [1, 9]]),
                            in_=AP(idxt.tensor,
                                   idxt[:].offset + (h * 4 + bl) * 9,
                                   [[NBLK * 9, 128], [1, 9]]))
                    idxw = gpool.tile([16, HTOK // 16], i16, tag="idxw")
                    vec.tensor_copy(
                        AP(idxw.tensor, idxw[:].offset,
                           [[HTOK // 16, 16], [32, 9], [1, 32]]),
                        AP(idxm.tensor, idxm[:].offset,
                           [[HTOK // 16, 16], [1, 9], [9, 32]]))
                    idxr = cpool.tile([128, HTOK // 16], i16,
                                      tag=f"idxr{g}{h}", name=f"idxr{g}{h}")
                    snc.dma_start(out=idxr[0:16, :], in_=idxw[:])
                    snc.dma_start(out=idxr[16:32, :], in_=idxw[:])
                    snc.dma_start(out=idxr[32:64, :], in_=idxr[0:32, :])
                    snc.dma_start(out=idxr[64:128, :], in_=idxr[0:64, :])
                    idxr_t[(g, h)] = idxr

            act.activation(MASK[s, :], V[s, :], ACTF.Abs, bias=mbias[s, :],
                           scale=1.0)
            vec.tensor_scalar(MASK[s, :], MASK[s, :], 63.5, None, ALU.is_gt)
            vec.tensor_tensor(MASK[s, :], MASK[s, :], FRAC[s, :], op=ALU.mult)
            vec.tensor_tensor(V[s, :], V[s, :], MASK[s, :], op=ALU.subtract)
            vec.tensor_scalar(V[s, :], V[s, :], 0.0, Hp - 1.0, ALU.max, ALU.min)
            vec.scalar_tensor_tensor(GLT[s, :], QLT[s, :], 1.0, V[s, :],
                                     op0=ALU.add, op1=ALU.subtract)
            vec.scalar_tensor_tensor(GRB[s, :], V[s, :], 1.0, QRB[s, :],
                                     op0=ALU.add, op1=ALU.subtract)
            # modulation mm = m * ad_m (valid on x-rows g*32..+9)
            vec.tensor_scalar(ADM[s, :], ADF[s, :], 2.0, -4.0, ALU.mult, ALU.add)
            vec.tensor_tensor(ADM[s, :], PSGM[s, :], ADM[s, :], op=ALU.mult)
            vec.tensor_tensor(T1[s, :], ADM[s, :], GLT[s, :], op=ALU.mult)
            vec.tensor_tensor(T2[s, :], ADM[s, :], GRB[s, :], op=ALU.mult)
            for g in range(NG):
                b0 = g * 32
                snc.dma_start(out=GLTY[b0:b0 + 9, :], in_=GLT[b0 + 9:b0 + 18, :])
                snc.dma_start(out=GRBY[b0:b0 + 9, :], in_=GRB[b0 + 9:b0 + 18, :])
            for (srcw, gy, tag) in ((T1, GLTY, "w00"), (T1, GRBY, "w01"),
                                    (T2, GLTY, "w10"), (T2, GRBY, "w11")):
                vec.tensor_tensor(WCT[tag][s, :], srcw[s, :], gy[s, :],
                                  op=ALU.mult)

        # ================= phase D: gather/apply/matmul pipeline ==========
        for g in range(NG):
            b0 = g * 32
            for h in range(2):
                idxr = idxr_t[(g, h)]
                # gather (non-transpose, 512B patch tokens, 2 queues)
                Gp = gpool.tile([128, 36 * 256], bf16, tag="Gp")
                for qh in range(2):
                    qn = (2 * (g * 2 + h) + qh) % 4
                    gp.dma_gather(
                        out_ap=AP(Gp.tensor, Gp[:].offset + qh * 18 * 256,
                                  [[36 * 256, 128], [256, 18], [1, 256]]),
                        in_ap=xe[:],
                        idxs_ap=idxr[:, qh * (QTOK // 16):(qh + 1) * (QTOK // 16)],
                        num_idxs=QTOK,
                        num_idxs_reg=QTOK,
                        elem_size=256,
                        elem_step=256,
                        transpose=False,
                        single_packet=False,
                        queue_num=qn)

                # xbar transpose to channel-major, split per queue-gather so
                # the first half overlaps the second gather:
                # G2[q, b*256 + r*128 + p] = Gp[p, b*256 + r*128 + q]
                G2 = g2pool.tile([128, 2 * HTOK], bf16, tag="G2")
                teng = snc if (g * 2 + h) % 2 else act
                for qh in range(2):
                    teng.dma_start(
                        out=AP(G2.tensor, G2[:].offset + qh * HTOK,
                               [[2 * HTOK, 128], [128, 36], [1, 128]]),
                        in_=Gp[:, qh * 18 * 256:(qh + 1) * 18 * 256],
                        transpose=True)

                # corner-weight broadcast (ACT HWDGE pipe, off the Sync path):
                # WRB[2c+pp, (n*4+ch)*256 + r*128 + p] = wc_{r,pp}[n, ch*128+p]
                # seed rows {0,1} then double 2 -> 128.
                WRB = wrbpool.tile([128, 2 * HTOK], bf16, tag="wrb")
                PIT = 2 * HTOK
                weng = act if (g * 2 + h) % 2 else snc
                for r_ in range(2):
                    for pp in range(2):
                        weng.dma_start(
                            out=AP(WRB.tensor,
                                   WRB[:].offset + pp * PIT + r_ * 128,
                                   [[PIT, 1], [1024, 9], [256, 4], [1, 128]]),
                            in_=WCL[r_ * 2 + pp][b0:b0 + 9,
                                                 h * 512:(h + 1) * 512])
                for kk in (2, 4, 8, 16, 32, 64):
                    weng.dma_start(
                        out=AP(WRB.tensor, WRB[:].offset + kk * PIT,
                               [[PIT, kk], [1, PIT]]),
                        in_=AP(WRB.tensor, WRB[:].offset,
                               [[PIT, kk], [1, PIT]]))

                # apply weights (in place, contiguous)
                for hh in range(2):
                    slh = slice(hh * HTOK, (hh + 1) * HTOK)
                    any_.tensor_tensor(G2[:, slh], G2[:, slh], WRB[:, slh],
                                       op=ALU.mult)

                # final matmuls
                po = psA.tile([64, 512], f32, tag="po")
                for tt in range(18):
                    r_, n_ = tt // 9, tt % 9
                    rhs = AP(G2.tensor,
                             G2[:].offset + n_ * 1024 + r_ * 128,
                             [[2 * HTOK, 128], [256, 4], [1, 128]])
                    pe.matmul(po[:], w3b[:, tt * 64:(tt + 1) * 64], rhs,
                              start=(tt == 0), stop=(tt == 17))
                oc = wpool.tile([64, 512], f32, tag="oc")
                vec.tensor_copy(oc[:], po[:])
                snc.dma_start(
                    out=t['outp'][:, g * GPOS + h * 512:g * GPOS + (h + 1) * 512],
                    in_=oc[:])


def _build():
    import concourse.bacc as bacc
    import concourse.tile as tile
    import concourse.mybir as mybir
    dt = mybir.dt

    nc = bacc.Bacc("TRN2", target_bir_lowering=False, debug=False,
                   num_swdge_queues=4)
    t = {}
    specs = [
        ('xs', [C, HSH + 2, W], dt.float32),
        ('xe', [NPATCH, 256], dt.bfloat16),
        ('wt', [128, 6 * M_CONV], dt.float32),
        ('bp128', [128, 1], dt.float32),
        ('pn128', [128, 1], dt.float32),
        ('lconst', [128, 9], dt.float32),
        ('w3', [128, 18 * 64], dt.bfloat16),
        ('ident', [128, 128], dt.float32),
        ('basep', [128, GPOS], dt.float32),
    ]
    for name, shape, d in specs:
        t[name] = nc.dram_tensor(name, shape, d, kind="ExternalInput").ap()
    t['outp'] = nc.dram_tensor('outp', [64, NPOS], dt.float32,
                               kind="ExternalOutput").ap()
    with tile.TileContext(nc) as tc:
        _emit(nc, tc, t)
    nc.compile()
    return nc


def kernel(x, w_p, b_p, w_m, w_ad, w_conv):
    from concourse.bass_utils import run_bass_kernel_spmd

    x = np.asarray(x, np.float32)
    consts = _prep_consts(np.asarray(w_p, np.float32), np.asarray(b_p, np.float32),
                          np.asarray(w_m, np.float32), np.asarray(w_ad, np.float32),
                          np.asarray(w_conv, np.float32))
    for b in range(B):
        _cache[('xe', b)] = _prep_table(x[b])
    if 'nc' not in _cache:
        _cache['nc'] = _build()
    nc = _cache['nc']

    in_maps = [_prep_core_inputs(c, x, consts) for c in range(NCORES)]
    res = run_bass_kernel_spmd(nc, in_maps, list(range(NCORES)))
    _cache['last_results'] = res

    out = np.zeros((B, 64, H, W), np.float32)
    for c in range(NCORES):
        b, hc = c // 4, c % 4
        out[b, :, hc * HSH:(hc + 1) * HSH, :] = \
            res.results[c]['outp'].reshape(64, HSH, W)
    return out


# revision 41
# speedup vs baseline: 1.1401x; 1.1401x over previous
"""Deformable Conv2d (adaptive, modulated) for Trainium2 — 8-core SPMD Bass kernel.

Strategy
--------
Shard (batch, H) into 8 shards: core = b*4 + hchunk, each computes 32 output
rows of one batch element.

Per core pipeline:
 1. Fused 3x3 conv (PE, f32) per group of 1024 positions produces offsets /
    adaptive-dilation / modulation rows; results are DMA-rebased into
    partition-packed [128, 1024] tiles (group g in rows g*32..g*32+17) so the
    whole coordinate stage runs once on all 128 DVE lanes instead of 18.
 2. f32 coordinate math (DVE/ACT) reproduces the reference's floor/clamp/mask
    bilinear weights exactly; one patch index u*131+v per (n,pos) with
    u,v = clamp(floor(p)+1, 0, 130) into a host-built edge-replicated
    132x132 "2x2 patch table" (entry = 4 corner pixels x 64 ch, 512 B).
 3. dma_gather (non-transpose mode, bf16, 512B tokens) runs descriptor
    generation on 4 SWDGE queues round-robin = 4 concurrent Q7 core pairs
    (transpose-mode sprays share xbar state and cannot overlap; plain CME
    writes can). Tokens land position-major.
 4. One HWDGE xbar DMA-transpose per half-group flips to channel-major
    G2[(pp,c), b*256 + r*128 + p] — matmul-ready.
 5. Corner weights mm*gx*gy are seeded in matching interleaved order,
    partition-broadcast by doubling DMAs, applied with two contiguous bf16
    tensor_tensor multiplies in place.
 6. The 4-corner bilinear sum AND the final 3x3 (stride-3) conv collapse
    into 18 PE matmuls with K=(pp,c)=128 per (r,n) tile (f32 PSUM accum).

The gather table, weight repacks, and coordinate base planes are prepared on
the host in numpy (layout/sharding prep only — no FLOPs on tensor data other
than the bf16 cast of the table).
"""

import numpy as np
import ml_dtypes

# ---- problem constants (hardcoded per contract) ----
B, C, H, W = 2, 64, 128, 128
KS, N, DIL, PAD = 3, 9, 2, 1
Hp = H + 2 * PAD            # 130
EXT = Hp + 2                # 132 (edge-replicated ext rows/cols)
NPATCH = 131 * 131          # patch-table entries (2x2 windows of ext image)
NCORES = 8
HSH = H // 4                # 32 rows per core
NPOS = HSH * W              # 4096 positions per core
NG = 4                      # groups per core
GPOS = NPOS // NG           # 1024 positions per group
NBLK = GPOS // 128          # 8 pos-blocks of 128 per group
M_CONV = 73                 # fused conv rows: off 0:18 | ad 32:50 | m 64:73
HPOS = 512                  # positions per half-group
HTOK = HPOS * N             # 4608 patch tokens per half-group
QTOK = HTOK // 2            # 2304 tokens per queue-gather

_cache = {}


# ======================================================================
# host-side input preparation
# ======================================================================

def _prep_consts(w_p, b_p, w_m, w_ad, w_conv):
    f32 = np.float32
    # fused conv taps: wt[t, c, m], t = dy*3+dx
    wt = np.zeros((9, C, M_CONV), f32)
    rep3 = [0, 1, 2, 0, 1, 2, 0, 1, 2]
    for t in range(9):
        dy, dx = t // 3, t % 3
        wt[t, :, 0:9] = w_p[0:9, :, dy, dx].T
        wt[t, :, 9:18] = w_p[9:18, :, dy, dx].T
        wt[t, :, 32:41] = w_ad[rep3, :, dy, dx].T
        wt[t, :, 41:50] = w_ad[rep3, :, dy, dx].T
        wt[t, :, 64:73] = w_m[0:9, :, dy, dx].T
    # tap-paired layout: tile m<3 pairs taps (m,0)+(m,1) over K=128 (the
    # second operand half reads the +1-column-shifted slab copy); tile 3+m
    # holds tap (m,2) over K=64.
    wtp = np.zeros((128, 6 * M_CONV), f32)
    for dy in range(3):
        wtp[0:64, dy * M_CONV:(dy + 1) * M_CONV] = wt[dy * 3 + 0]
        wtp[64:128, dy * M_CONV:(dy + 1) * M_CONV] = wt[dy * 3 + 1]
        wtp[0:64, (3 + dy) * M_CONV:(4 + dy) * M_CONV] = wt[dy * 3 + 2]
    wt = wtp

    # packed per-partition constants: group g occupies rows g*32 .. g*32+17
    bp128 = np.zeros((128, 1), f32)
    pn128 = np.zeros((128, 1), f32)
    r = np.array([-1.0, 0.0, 1.0], f32)
    pnx = np.repeat(r, 3)
    pny = np.tile(r, 3)
    pn18 = np.concatenate([pnx, pny]).astype(f32)
    for g in range(NG):
        bp128[g * 32:g * 32 + 18, 0] = b_p
        pn128[g * 32:g * 32 + 18, 0] = pn18

    # idx matmul matrix, replicated per 32-row band: idx = 131*R1x + R1y
    lconst = np.zeros((128, 9), f32)
    for g in range(NG):
        for n in range(9):
            lconst[g * 32 + n, n] = 131.0
            lconst[g * 32 + 9 + n, n] = 1.0
    # w3: [128=(c,pp): row 2c+pp], 18*64] bf16, k-tile t=(r*9+n)
    w3 = np.zeros((128, 18 * 64), f32)
    for t in range(18):
        n = t % 9
        blk = w_conv[:, :, n // 3, n % 3].T  # [c, o]
        w3[0::2, t * 64:(t + 1) * 64] = blk
        w3[1::2, t * 64:(t + 1) * 64] = blk
    w3 = w3.astype(ml_dtypes.bfloat16)

    ident = np.eye(128, dtype=f32)
    return dict(wt=wt, bp128=bp128, pn128=pn128, lconst=lconst, w3=w3,
                ident=ident)


def _prep_table(xb):
    """xb: [C, H, W] f32 -> patch table [NPATCH, 256] bf16.

    Entry (u, v) (u, v in [0, 130], idx = u*131+v) holds the 2x2 pixel
    patch of the edge-replicated 132x132 padded image at rows (u, u+1),
    cols (v, v+1), laid out (r, pp, c)."""
    xp = np.pad(xb, ((0, 0), (PAD, PAD), (PAD, PAD)))          # [C, 130, 130]
    idx = np.clip(np.arange(EXT) - 1, 0, Hp - 1)
    ext = xp[:, idx][:, :, idx]                                # [C, 132, 132]
    win = np.lib.stride_tricks.sliding_window_view(ext, (2, 2), axis=(1, 2))
    # win: [C, 131, 131, 2, 2] -> (u, v, r, c, pp): the (c,pp) interleave
    # lets the corner-weight broadcast seed a row pair and double 2->128.
    patch = np.ascontiguousarray(win.transpose(1, 2, 3, 0, 4)).reshape(NPATCH, 256)
    return patch.astype(ml_dtypes.bfloat16)


def _prep_core_inputs(core, x, consts):
    b, hc = core // 4, core % 4
    h0 = hc * HSH
    # conv input rows h0-1 .. h0+32 (34 rows), zero padded at batch edges
    xs = np.zeros((C, HSH + 2, W), np.float32)
    lo, hi = h0 - 1, h0 + HSH + 1
    slo, shi = max(lo, 0), min(hi, H)
    xs[:, slo - lo:shi - lo, :] = x[b, :, slo:shi, :]

    # packed base coords: rows g*32+j (j<18), cols = pos within group
    basep = np.zeros((128, GPOS), np.float32)
    pos = np.arange(NPOS)
    bx = (h0 + pos // W + 1).astype(np.float32)
    by = (pos % W + 1).astype(np.float32)
    for g in range(NG):
        sl = slice(g * GPOS, (g + 1) * GPOS)
        basep[g * 32:g * 32 + 9, :] = bx[sl][None, :]
        basep[g * 32 + 9:g * 32 + 18, :] = by[sl][None, :]

    m = dict(xs=xs, basep=basep, xe=_cache[('xe', b)])
    m.update({k: consts[k] for k in ('wt', 'bp128', 'pn128', 'lconst', 'w3',
                                     'ident')})
    return m


# ======================================================================
# bass program
# ======================================================================

def _emit(nc, tc, t):
    import concourse.bass as bass
    import concourse.mybir as mybir
    from concourse.bass import AP

    dt = mybir.dt
    ALU = mybir.AluOpType
    ACTF = mybir.ActivationFunctionType
    f32, bf16, i16, i32 = dt.float32, dt.bfloat16, dt.int16, dt.int32

    XROW = HSH + 2          # 34
    XCW = W + 2             # 130 padded row width in sbuf
    any_, vec, act, pe, gp, snc = nc.any, nc.vector, nc.scalar, nc.tensor, nc.gpsimd, nc.sync

    with tc.tile_pool(name="const", bufs=1) as cpool, \
         tc.tile_pool(name="work", bufs=1) as wpool, \
         tc.tile_pool(name="gath", bufs=2) as gpool, \
         tc.tile_pool(name="gath2", bufs=2) as g2pool, \
         tc.tile_pool(name="wrbp", bufs=2) as wrbpool, \
         tc.tile_pool(name="psA", bufs=2, space="PSUM") as psA, \
         tc.tile_pool(name="psB", bufs=2, space="PSUM") as psB:
        kpool = wpool

        # ---- load constants / inputs to SBUF ----
        xsb = cpool.tile([128, XROW * XCW], f32, tag="xsb")
        gp.memset(xsb[:], 0.0)
        snc.dma_start(
            out=AP(xsb.tensor, xsb[:].offset + 1,
                   [[XROW * XCW, C], [XCW, XROW], [1, W]]),
            in_=t['xs'][:])
        snc.dma_start(
            out=AP(xsb.tensor, xsb[:].offset + 64 * (XROW * XCW),
                   [[XROW * XCW, C], [XCW, XROW], [1, W]]),
            in_=t['xs'][:])
        wtb = cpool.tile([128, 6 * M_CONV], f32, tag="wtb")
        snc.dma_start(out=wtb[:], in_=t['wt'][:])
        w3b = cpool.tile([128, 18 * 64], bf16, tag="w3b")
        snc.dma_start(out=w3b[:], in_=t['w3'][:])
        idb = cpool.tile([18, 18], f32, tag="idb")
        snc.dma_start(out=idb[:], in_=t['ident'][:18, :18])
        bpb = cpool.tile([128, 1], f32, tag="bpb")
        snc.dma_start(out=bpb[:], in_=t['bp128'][:])
        pnb = cpool.tile([128, 1], f32, tag="pnb")
        snc.dma_start(out=pnb[:], in_=t['pn128'][:])
        lcb = cpool.tile([128, 9], f32, tag="lcb")
        snc.dma_start(out=lcb[:], in_=t['lconst'][:])
        mbias = cpool.tile([128, 1], f32, tag="mbias")
        gp.memset(mbias[:], -64.5)
        bslp = cpool.tile([128, GPOS], f32, tag="bslp")
        snc.dma_start(out=bslp[:], in_=t['basep'][:])

        xe = t['xe']  # dram [NPATCH, 256] bf16

        def pt(tag):
            return kpool.tile([128, GPOS], f32, tag=tag, name=tag)

        # ================= phase A: convs -> packed tiles =================
        POFF, PSGN, PSGM = pt("poff"), pt("psgn"), pt("psgm")
        for g in range(NG):
            conv = kpool.tile([M_CONV, GPOS], f32, tag="conv")
            for k in range(2):
                pc = psA.tile([M_CONV, 512], f32, tag="pc")
                for dy in range(3):
                    rhs = AP(xsb.tensor,
                             xsb[:].offset + (g * 8 + k * 4 + dy) * XCW,
                             [[XROW * XCW, 128], [XCW, 4], [1, W]])
                    pe.matmul(pc[:], wtb[:, dy * M_CONV:(dy + 1) * M_CONV],
                              rhs, start=(dy == 0), stop=False)
                for dy in range(3):
                    rhs = AP(xsb.tensor,
                             xsb[:].offset + (g * 8 + k * 4 + dy) * XCW + 2,
                             [[XROW * XCW, C], [XCW, 4], [1, W]])
                    pe.matmul(pc[:],
                              wtb[0:64, (3 + dy) * M_CONV:(4 + dy) * M_CONV],
                              rhs, start=False, stop=(dy == 2))
                sl = slice(k * 512, (k + 1) * 512)
                act.activation(conv[0:18, sl], pc[0:18, :], ACTF.Identity,
                               bias=bpb[0:18, :], scale=1.0)
                act.activation(conv[32:50, sl], pc[32:50, :], ACTF.Sigmoid,
                               scale=-1.0)
                act.activation(conv[64:73, sl], pc[64:73, :], ACTF.Sigmoid,
                               scale=1.0)
            b0 = g * 32
            snc.dma_start(out=POFF[b0:b0 + 18, :], in_=conv[0:18, :])
            snc.dma_start(out=PSGN[b0:b0 + 18, :], in_=conv[32:50, :])
            snc.dma_start(out=PSGM[b0:b0 + 9, :], in_=conv[64:73, :])

        # ======== phase B/B': coordinates + idx prep, per 64-row half ======
        idxr_t = {}
        V = pt("v")
        I32T = kpool.tile([128, GPOS], i32, tag="i32t")
        F = pt("f")
        FRAC = pt("frac")
        QRB1 = pt("qrb1")
        R1 = pt("r1")
        MASK = pt("mask")
        GLT = kpool.tile([128, GPOS], f32, tag="poff", name="glt")
        GRB = kpool.tile([128, GPOS], f32, tag="mask", name="grb")
        ADM = kpool.tile([128, GPOS], f32, tag="v", name="adm")
        T1 = kpool.tile([128, GPOS], f32, tag="frac", name="t1")
        T2 = kpool.tile([128, GPOS], f32, tag="i32t", name="t2")
        GLTY = kpool.tile([128, GPOS], f32, tag="psgm", name="glty")
        GRBY = kpool.tile([128, GPOS], f32, tag="qrb1", name="grby")
        WCT = {}
        for tag in ("w00", "w01", "w10", "w11"):
            WCT[tag] = wpool.tile([128, GPOS], bf16, tag=tag, name=tag)
        WCL = [WCT["w00"], WCT["w01"], WCT["w10"], WCT["w11"]]

        for sg in range(1):
            s = slice(0, 128)
            # ADF = 1 + 2*(1-sigmoid), in place over PSGN
            vec.tensor_scalar(PSGN[s, :], PSGN[s, :], 2.0, 1.0, ALU.mult, ALU.add)
            ADF = PSGN
            vec.tensor_tensor(V[s, :], POFF[s, :], bslp[s, :], op=ALU.add)
            vec.scalar_tensor_tensor(V[s, :], ADF[s, :], pnb[s, :], V[s, :],
                                     op0=ALU.mult, op1=ALU.add)
            # floor(V) robust to convert rounding: g=int(V); F=g-(g>V)
            vec.tensor_copy(I32T[s, :], V[s, :])
            vec.tensor_copy(F[s, :], I32T[s, :])
            vec.tensor_tensor(FRAC[s, :], F[s, :], V[s, :], op=ALU.is_gt)
            vec.tensor_tensor(F[s, :], F[s, :], FRAC[s, :], op=ALU.subtract)
            vec.tensor_tensor(FRAC[s, :], V[s, :], F[s, :], op=ALU.subtract)
            vec.tensor_scalar(QRB1[s, :], F[s, :], 1.0, 0.0, ALU.add, ALU.max)
            vec.tensor_scalar(F[s, :], F[s, :], 0.0, Hp - 1.0, ALU.max, ALU.min)
            QLT = F
            vec.tensor_scalar(R1[s, :], QRB1[s, :], Hp + 0.0, None, ALU.min)
            vec.tensor_scalar(QRB1[s, :], QRB1[s, :], Hp - 1.0, None, ALU.min)
            QRB = QRB1
            act.activation(MASK[s, :], V[s, :], ACTF.Abs, bias=mbias[s, :],
                           scale=1.0)
            vec.tensor_scalar(MASK[s, :], MASK[s, :], 63.5, None, ALU.is_gt)
            vec.tensor_tensor(MASK[s, :], MASK[s, :], FRAC[s, :], op=ALU.mult)
            vec.tensor_tensor(V[s, :], V[s, :], MASK[s, :], op=ALU.subtract)
            vec.tensor_scalar(V[s, :], V[s, :], 0.0, Hp - 1.0, ALU.max, ALU.min)
            vec.scalar_tensor_tensor(GLT[s, :], QLT[s, :], 1.0, V[s, :],
                                     op0=ALU.add, op1=ALU.subtract)
            vec.scalar_tensor_tensor(GRB[s, :], V[s, :], 1.0, QRB[s, :],
                                     op0=ALU.add, op1=ALU.subtract)
            # modulation mm = m * ad_m (valid on x-rows g*32..+9)
            vec.tensor_scalar(ADM[s, :], ADF[s, :], 2.0, -4.0, ALU.mult, ALU.add)
            vec.tensor_tensor(ADM[s, :], PSGM[s, :], ADM[s, :], op=ALU.mult)
            vec.tensor_tensor(T1[s, :], ADM[s, :], GLT[s, :], op=ALU.mult)
            vec.tensor_tensor(T2[s, :], ADM[s, :], GRB[s, :], op=ALU.mult)
            for g in range(NG):
                b0 = g * 32
                snc.dma_start(out=GLTY[b0:b0 + 9, :], in_=GLT[b0 + 9:b0 + 18, :])
                snc.dma_start(out=GRBY[b0:b0 + 9, :], in_=GRB[b0 + 9:b0 + 18, :])
            for (srcw, gy, tag) in ((T1, GLTY, "w00"), (T1, GRBY, "w01"),
                                    (T2, GLTY, "w10"), (T2, GRBY, "w11")):
                vec.tensor_tensor(WCT[tag][s, :], srcw[s, :], gy[s, :],
                                  op=ALU.mult)

            for g in range(NG):
                b0 = g * 32
                # idxf col (bl*128 + p*8 + s) <- idx[n, pos=bl*128+s*16+p]
                idxf = kpool.tile([9, GPOS], f32, tag="idxf")
                for k in range(2):
                    pi = psB.tile([9, 512], f32, tag="pi")
                    pe.matmul(pi[:], lcb[b0:b0 + 18, :],
                              R1[b0:b0 + 18, k * 512:(k + 1) * 512],
                              start=True, stop=True, tile_position=(b0, 0))
                    act.activation(
                        AP(idxf.tensor, idxf[:].offset + k * 512,
                           [[GPOS, 9], [128, 4], [1, 8], [8, 16]]),
                        pi[:], ACTF.Copy, scale=1.0)
                # idxt[p*8+s, bl*9+n] = idx[n, pos=bl*128+s*16+p] (i16)
                idxt = kpool.tile([128, NBLK * 9], i16, tag="idxt")
                for bl in range(NBLK):
                    tp = psB.tile([128, 9], f32, tag="tp")
                    pe.transpose(tp[:], idxf[:, bl * 128:(bl + 1) * 128],
                                 idb[0:9, 0:9])
                    vec.tensor_copy(idxt[:, bl * 9:bl * 9 + 9], tp[:])

                for h in range(2):
                    idxm = gpool.tile([16, HTOK // 16], i16, tag="idxm")
                    for bl in range(4):
                        act.dma_start(
                            out=AP(idxm.tensor, idxm[:].offset + bl * 72,
                                   [[HTOK // 16, 16], [9, 8], [1, 9]]),
                            in_=AP(idxt.tensor,
                                   idxt[:].offset + (h * 4 + bl) * 9,
                                   [[NBLK * 9, 128], [1, 9]]))
                    idxw = gpool.tile([16, HTOK // 16], i16, tag="idxw")
                    vec.tensor_copy(
                        AP(idxw.tensor, idxw[:].offset,
                           [[HTOK // 16, 16], [32, 9], [1, 32]]),
                        AP(idxm.tensor, idxm[:].offset,
                           [[HTOK // 16, 16], [1, 9], [9, 32]]))
                    idxr = cpool.tile([128, HTOK // 16], i16,
                                      tag=f"idxr{g}{h}", name=f"idxr{g}{h}")
                    snc.dma_start(out=idxr[0:16, :], in_=idxw[:])
                    snc.dma_start(out=idxr[16:32, :], in_=idxw[:])
                    snc.dma_start(out=idxr[32:64, :], in_=idxr[0:32, :])
                    snc.dma_start(out=idxr[64:128, :], in_=idxr[0:64, :])
                    idxr_t[(g, h)] = idxr

        # ================= phase D: gather/apply/matmul pipeline ==========
        for g in range(NG):
            b0 = g * 32
            for h in range(2):
                idxr = idxr_t[(g, h)]
                # gather (non-transpose, 512B patch tokens, 2 queues)
                Gp = gpool.tile([128, 36 * 256], bf16, tag="Gp")
                for qh in range(2):
                    qn = (2 * (g * 2 + h) + qh) % 4
                    gp.dma_gather(
                        out_ap=AP(Gp.tensor, Gp[:].offset + qh * 18 * 256,
                                  [[36 * 256, 128], [256, 18], [1, 256]]),
                        in_ap=xe[:],
                        idxs_ap=idxr[:, qh * (QTOK // 16):(qh + 1) * (QTOK // 16)],
                        num_idxs=QTOK,
                        num_idxs_reg=QTOK,
                        elem_size=256,
                        elem_step=256,
                        transpose=False,
                        single_packet=False,
                        queue_num=qn)

                # xbar transpose to channel-major, split per queue-gather so
                # the first half overlaps the second gather:
                # G2[q, b*256 + r*128 + p] = Gp[p, b*256 + r*128 + q]
                G2 = g2pool.tile([128, 2 * HTOK], bf16, tag="G2")
                teng = snc if (g * 2 + h) % 2 else act
                for qh in range(2):
                    teng.dma_start(
                        out=AP(G2.tensor, G2[:].offset + qh * HTOK,
                               [[2 * HTOK, 128], [128, 36], [1, 128]]),
                        in_=Gp[:, qh * 18 * 256:(qh + 1) * 18 * 256],
                        transpose=True)

                # corner-weight broadcast (ACT HWDGE pipe, off the Sync path):
                # WRB[2c+pp, (n*4+ch)*256 + r*128 + p] = wc_{r,pp}[n, ch*128+p]
                # seed rows {0,1} then double 2 -> 128.
                WRB = wrbpool.tile([128, 2 * HTOK], bf16, tag="wrb")
                PIT = 2 * HTOK
                weng = act if (g * 2 + h) % 2 else snc
                for r_ in range(2):
                    for pp in range(2):
                        weng.dma_start(
                            out=AP(WRB.tensor,
                                   WRB[:].offset + pp * PIT + r_ * 128,
                                   [[PIT, 1], [1024, 9], [256, 4], [1, 128]]),
                            in_=WCL[r_ * 2 + pp][b0:b0 + 9,
                                                 h * 512:(h + 1) * 512])
                for kk in (2, 4, 8, 16, 32, 64):
                    weng.dma_start(
                        out=AP(WRB.tensor, WRB[:].offset + kk * PIT,
                               [[PIT, kk], [1, PIT]]),
                        in_=AP(WRB.tensor, WRB[:].offset,
                               [[PIT, kk], [1, PIT]]))

                # apply weights (in place, contiguous)
                for hh in range(2):
                    slh = slice(hh * HTOK, (hh + 1) * HTOK)
                    any_.tensor_tensor(G2[:, slh], G2[:, slh], WRB[:, slh],
                                       op=ALU.mult)

                # final matmuls
                po = psA.tile([64, 512], f32, tag="po")
                for tt in range(18):
                    r_, n_ = tt // 9, tt % 9
                    rhs = AP(G2.tensor,
                             G2[:].offset + n_ * 1024 + r_ * 128,
                             [[2 * HTOK, 128], [256, 4], [1, 128]])
                    pe.matmul(po[:], w3b[:, tt * 64:(tt + 1) * 64], rhs,
                              start=(tt == 0), stop=(tt == 17))
                oc = wpool.tile([64, 512], f32, tag="oc")
                vec.tensor_copy(oc[:], po[:])
                snc.dma_start(
                    out=t['outp'][:, g * GPOS + h * 512:g * GPOS + (h + 1) * 512],
                    in_=oc[:])


def _build():
    import concourse.bacc as bacc
    import concourse.tile as tile
    import concourse.mybir as mybir
    dt = mybir.dt

    nc = bacc.Bacc("TRN2", target_bir_lowering=False, debug=False,
                   num_swdge_queues=4)
    t = {}
    specs = [
        ('xs', [C, HSH + 2, W], dt.float32),
        ('xe', [NPATCH, 256], dt.bfloat16),
        ('wt', [128, 6 * M_CONV], dt.float32),
        ('bp128', [128, 1], dt.float32),
        ('pn128', [128, 1], dt.float32),
        ('lconst', [128, 9], dt.float32),
        ('w3', [128, 18 * 64], dt.bfloat16),
        ('ident', [128, 128], dt.float32),
        ('basep', [128, GPOS], dt.float32),
    ]
    for name, shape, d in specs:
        t[name] = nc.dram_tensor(name, shape, d, kind="ExternalInput").ap()
    t['outp'] = nc.dram_tensor('outp', [64, NPOS], dt.float32,
                               kind="ExternalOutput").ap()
    with tile.TileContext(nc) as tc:
        _emit(nc, tc, t)
    nc.compile()
    return nc


def kernel(x, w_p, b_p, w_m, w_ad, w_conv):
    from concourse.bass_utils import run_bass_kernel_spmd

    x = np.asarray(x, np.float32)
    consts = _prep_consts(np.asarray(w_p, np.float32), np.asarray(b_p, np.float32),
                          np.asarray(w_m, np.float32), np.asarray(w_ad, np.float32),
                          np.asarray(w_conv, np.float32))
    for b in range(B):
        _cache[('xe', b)] = _prep_table(x[b])
    if 'nc' not in _cache:
        _cache['nc'] = _build()
    nc = _cache['nc']

    in_maps = [_prep_core_inputs(c, x, consts) for c in range(NCORES)]
    res = run_bass_kernel_spmd(nc, in_maps, list(range(NCORES)))
    _cache['last_results'] = res

    out = np.zeros((B, 64, H, W), np.float32)
    for c in range(NCORES):
        b, hc = c // 4, c % 4
        out[b, :, hc * HSH:(hc + 1) * HSH, :] = \
            res.results[c]['outp'].reshape(64, HSH, W)
    return out


# revision 42
# speedup vs baseline: 1.1582x; 1.0159x over previous
"""Deformable Conv2d (adaptive, modulated) for Trainium2 — 8-core SPMD Bass kernel.

Strategy
--------
Shard (batch, H) into 8 shards: core = b*4 + hchunk, each computes 32 output
rows of one batch element.

Per core pipeline:
 1. Fused 3x3 conv (PE, f32) per group of 1024 positions produces offsets /
    adaptive-dilation / modulation rows; results are DMA-rebased into
    partition-packed [128, 1024] tiles (group g in rows g*32..g*32+17) so the
    whole coordinate stage runs once on all 128 DVE lanes instead of 18.
 2. f32 coordinate math (DVE/ACT) reproduces the reference's floor/clamp/mask
    bilinear weights exactly; one patch index u*131+v per (n,pos) with
    u,v = clamp(floor(p)+1, 0, 130) into a host-built edge-replicated
    132x132 "2x2 patch table" (entry = 4 corner pixels x 64 ch, 512 B).
 3. dma_gather (non-transpose mode, bf16, 512B tokens) runs descriptor
    generation on 4 SWDGE queues round-robin = 4 concurrent Q7 core pairs
    (transpose-mode sprays share xbar state and cannot overlap; plain CME
    writes can). Tokens land position-major.
 4. One HWDGE xbar DMA-transpose per half-group flips to channel-major
    G2[(pp,c), b*256 + r*128 + p] — matmul-ready.
 5. Corner weights mm*gx*gy are seeded in matching interleaved order,
    partition-broadcast by doubling DMAs, applied with two contiguous bf16
    tensor_tensor multiplies in place.
 6. The 4-corner bilinear sum AND the final 3x3 (stride-3) conv collapse
    into 18 PE matmuls with K=(pp,c)=128 per (r,n) tile (f32 PSUM accum).

The gather table, weight repacks, and coordinate base planes are prepared on
the host in numpy (layout/sharding prep only — no FLOPs on tensor data other
than the bf16 cast of the table).
"""

import numpy as np
import ml_dtypes

# ---- problem constants (hardcoded per contract) ----
B, C, H, W = 2, 64, 128, 128
KS, N, DIL, PAD = 3, 9, 2, 1
Hp = H + 2 * PAD            # 130
EXT = Hp + 2                # 132 (edge-replicated ext rows/cols)
NPATCH = 131 * 131          # patch-table entries (2x2 windows of ext image)
NCORES = 8
HSH = H // 4                # 32 rows per core
NPOS = HSH * W              # 4096 positions per core
NG = 4                      # groups per core
GPOS = NPOS // NG           # 1024 positions per group
NBLK = GPOS // 128          # 8 pos-blocks of 128 per group
M_CONV = 73                 # fused conv rows: off 0:18 | ad 32:50 | m 64:73
HPOS = 512                  # positions per half-group
HTOK = HPOS * N             # 4608 patch tokens per half-group
QTOK = HTOK // 2            # 2304 tokens per queue-gather

_cache = {}


# ======================================================================
# host-side input preparation
# ======================================================================

def _prep_consts(w_p, b_p, w_m, w_ad, w_conv):
    f32 = np.float32
    # fused conv taps: wt[t, c, m], t = dy*3+dx
    wt = np.zeros((9, C, M_CONV), f32)
    rep3 = [0, 1, 2, 0, 1, 2, 0, 1, 2]
    for t in range(9):
        dy, dx = t // 3, t % 3
        wt[t, :, 0:9] = w_p[0:9, :, dy, dx].T
        wt[t, :, 9:18] = w_p[9:18, :, dy, dx].T
        wt[t, :, 32:41] = w_ad[rep3, :, dy, dx].T
        wt[t, :, 41:50] = w_ad[rep3, :, dy, dx].T
        wt[t, :, 64:73] = w_m[0:9, :, dy, dx].T
    # tap-paired layout: tile m<3 pairs taps (m,0)+(m,1) over K=128 (the
    # second operand half reads the +1-column-shifted slab copy); tile 3+m
    # holds tap (m,2) over K=64.
    wtp = np.zeros((128, 6 * M_CONV), f32)
    for dy in range(3):
        wtp[0:64, dy * M_CONV:(dy + 1) * M_CONV] = wt[dy * 3 + 0]
        wtp[64:128, dy * M_CONV:(dy + 1) * M_CONV] = wt[dy * 3 + 1]
        wtp[0:64, (3 + dy) * M_CONV:(4 + dy) * M_CONV] = wt[dy * 3 + 2]
    wt = wtp

    # packed per-partition constants: group g occupies rows g*32 .. g*32+17
    bp128 = np.zeros((128, 1), f32)
    pn128 = np.zeros((128, 1), f32)
    r = np.array([-1.0, 0.0, 1.0], f32)
    pnx = np.repeat(r, 3)
    pny = np.tile(r, 3)
    pn18 = np.concatenate([pnx, pny]).astype(f32)
    for g in range(NG):
        bp128[g * 32:g * 32 + 18, 0] = b_p
        pn128[g * 32:g * 32 + 18, 0] = pn18

    # idx matmul matrix, replicated per 32-row band: idx = 131*R1x + R1y
    lconst = np.zeros((128, 9), f32)
    for g in range(NG):
        for n in range(9):
            lconst[g * 32 + n, n] = 131.0
            lconst[g * 32 + 9 + n, n] = 1.0
    # w3: [128=(c,pp): row 2c+pp], 18*64] bf16, k-tile t=(r*9+n)
    w3 = np.zeros((128, 18 * 64), f32)
    for t in range(18):
        n = t % 9
        blk = w_conv[:, :, n // 3, n % 3].T  # [c, o]
        w3[0::2, t * 64:(t + 1) * 64] = blk
        w3[1::2, t * 64:(t + 1) * 64] = blk
    w3 = w3.astype(ml_dtypes.bfloat16)

    ident = np.eye(128, dtype=f32)
    return dict(wt=wt, bp128=bp128, pn128=pn128, lconst=lconst, w3=w3,
                ident=ident)


def _prep_table(xb):
    """xb: [C, H, W] f32 -> patch table [NPATCH, 256] bf16.

    Entry (u, v) (u, v in [0, 130], idx = u*131+v) holds the 2x2 pixel
    patch of the edge-replicated 132x132 padded image at rows (u, u+1),
    cols (v, v+1), laid out (r, pp, c)."""
    xp = np.pad(xb, ((0, 0), (PAD, PAD), (PAD, PAD)))          # [C, 130, 130]
    idx = np.clip(np.arange(EXT) - 1, 0, Hp - 1)
    ext = xp[:, idx][:, :, idx]                                # [C, 132, 132]
    win = np.lib.stride_tricks.sliding_window_view(ext, (2, 2), axis=(1, 2))
    # win: [C, 131, 131, 2, 2] -> (u, v, r, c, pp): the (c,pp) interleave
    # lets the corner-weight broadcast seed a row pair and double 2->128.
    patch = np.ascontiguousarray(win.transpose(1, 2, 3, 0, 4)).reshape(NPATCH, 256)
    return patch.astype(ml_dtypes.bfloat16)


def _prep_core_inputs(core, x, consts):
    b, hc = core // 4, core % 4
    h0 = hc * HSH
    # conv input rows h0-1 .. h0+32 (34 rows), zero padded at batch edges
    xs = np.zeros((C, HSH + 2, W), np.float32)
    lo, hi = h0 - 1, h0 + HSH + 1
    slo, shi = max(lo, 0), min(hi, H)
    xs[:, slo - lo:shi - lo, :] = x[b, :, slo:shi, :]

    # packed base coords: rows g*32+j (j<18), cols = pos within group
    basep = np.zeros((128, GPOS), np.float32)
    pos = np.arange(NPOS)
    bx = (h0 + pos // W + 1).astype(np.float32)
    by = (pos % W + 1).astype(np.float32)
    for g in range(NG):
        sl = slice(g * GPOS, (g + 1) * GPOS)
        basep[g * 32:g * 32 + 9, :] = bx[sl][None, :]
        basep[g * 32 + 9:g * 32 + 18, :] = by[sl][None, :]

    m = dict(xs=xs, basep=basep, xe=_cache[('xe', b)])
    m.update({k: consts[k] for k in ('wt', 'bp128', 'pn128', 'lconst', 'w3',
                                     'ident')})
    return m


# ======================================================================
# bass program
# ======================================================================

def _emit(nc, tc, t):
    import concourse.bass as bass
    import concourse.mybir as mybir
    from concourse.bass import AP

    dt = mybir.dt
    ALU = mybir.AluOpType
    ACTF = mybir.ActivationFunctionType
    f32, bf16, i16, i32 = dt.float32, dt.bfloat16, dt.int16, dt.int32

    XROW = HSH + 2          # 34
    XCW = W + 2             # 130 padded row width in sbuf
    any_, vec, act, pe, gp, snc = nc.any, nc.vector, nc.scalar, nc.tensor, nc.gpsimd, nc.sync

    with tc.tile_pool(name="const", bufs=1) as cpool, \
         tc.tile_pool(name="work", bufs=1) as wpool, \
         tc.tile_pool(name="gath", bufs=2) as gpool, \
         tc.tile_pool(name="gath2", bufs=2) as g2pool, \
         tc.tile_pool(name="wrbp", bufs=2) as wrbpool, \
         tc.tile_pool(name="psA", bufs=2, space="PSUM") as psA, \
         tc.tile_pool(name="psB", bufs=2, space="PSUM") as psB:
        kpool = wpool

        # ---- load constants / inputs to SBUF ----
        xsb = cpool.tile([128, XROW * XCW], f32, tag="xsb")
        gp.memset(xsb[:], 0.0)
        snc.dma_start(
            out=AP(xsb.tensor, xsb[:].offset + 1,
                   [[XROW * XCW, C], [XCW, XROW], [1, W]]),
            in_=t['xs'][:])
        snc.dma_start(
            out=AP(xsb.tensor, xsb[:].offset + 64 * (XROW * XCW),
                   [[XROW * XCW, C], [XCW, XROW], [1, W]]),
            in_=t['xs'][:])
        wtb = cpool.tile([128, 6 * M_CONV], f32, tag="wtb")
        snc.dma_start(out=wtb[:], in_=t['wt'][:])
        w3b = cpool.tile([128, 18 * 64], bf16, tag="w3b")
        snc.dma_start(out=w3b[:], in_=t['w3'][:])
        idb = cpool.tile([18, 18], f32, tag="idb")
        snc.dma_start(out=idb[:], in_=t['ident'][:18, :18])
        bpb = cpool.tile([128, 1], f32, tag="bpb")
        snc.dma_start(out=bpb[:], in_=t['bp128'][:])
        pnb = cpool.tile([128, 1], f32, tag="pnb")
        snc.dma_start(out=pnb[:], in_=t['pn128'][:])
        lcb = cpool.tile([128, 9], f32, tag="lcb")
        snc.dma_start(out=lcb[:], in_=t['lconst'][:])
        mbias = cpool.tile([128, 1], f32, tag="mbias")
        gp.memset(mbias[:], -64.5)
        bslp = cpool.tile([128, GPOS], f32, tag="bslp")
        snc.dma_start(out=bslp[:], in_=t['basep'][:])

        xe = t['xe']  # dram [NPATCH, 256] bf16

        def pt(tag):
            return kpool.tile([128, GPOS], f32, tag=tag, name=tag)

        # ================= phase A: convs -> packed tiles =================
        POFF, PSGN, PSGM = pt("poff"), pt("psgn"), pt("psgm")
        for g in range(NG):
            conv = kpool.tile([M_CONV, GPOS], f32, tag="conv")
            for k in range(2):
                pc = psA.tile([M_CONV, 512], f32, tag="pc")
                for dy in range(3):
                    rhs = AP(xsb.tensor,
                             xsb[:].offset + (g * 8 + k * 4 + dy) * XCW,
                             [[XROW * XCW, 128], [XCW, 4], [1, W]])
                    pe.matmul(pc[:], wtb[:, dy * M_CONV:(dy + 1) * M_CONV],
                              rhs, start=(dy == 0), stop=False)
                for dy in range(3):
                    rhs = AP(xsb.tensor,
                             xsb[:].offset + (g * 8 + k * 4 + dy) * XCW + 2,
                             [[XROW * XCW, C], [XCW, 4], [1, W]])
                    pe.matmul(pc[:],
                              wtb[0:64, (3 + dy) * M_CONV:(4 + dy) * M_CONV],
                              rhs, start=False, stop=(dy == 2))
                sl = slice(k * 512, (k + 1) * 512)
                act.activation(conv[0:18, sl], pc[0:18, :], ACTF.Identity,
                               bias=bpb[0:18, :], scale=1.0)
                act.activation(conv[32:50, sl], pc[32:50, :], ACTF.Sigmoid,
                               scale=-1.0)
                act.activation(conv[64:73, sl], pc[64:73, :], ACTF.Sigmoid,
                               scale=1.0)
            b0 = g * 32
            snc.dma_start(out=POFF[b0:b0 + 18, :], in_=conv[0:18, :])
            snc.dma_start(out=PSGN[b0:b0 + 18, :], in_=conv[32:50, :])
            snc.dma_start(out=PSGM[b0:b0 + 9, :], in_=conv[64:73, :])

        # ======== phase B/B': coordinates + idx prep, per 64-row half ======
        idxr_t = {}
        V = pt("v")
        I32T = kpool.tile([128, GPOS], i32, tag="i32t")
        F = pt("f")
        FRAC = pt("frac")
        QRB1 = pt("qrb1")
        R1 = pt("r1")
        MASK = pt("mask")
        GLT = kpool.tile([128, GPOS], f32, tag="poff", name="glt")
        GRB = kpool.tile([128, GPOS], f32, tag="mask", name="grb")
        ADM = kpool.tile([128, GPOS], f32, tag="v", name="adm")
        T1 = kpool.tile([128, GPOS], f32, tag="frac", name="t1")
        T2 = kpool.tile([128, GPOS], f32, tag="i32t", name="t2")
        GLTY = kpool.tile([128, GPOS], f32, tag="psgm", name="glty")
        GRBY = kpool.tile([128, GPOS], f32, tag="qrb1", name="grby")
        WCT = {}
        for tag in ("w00", "w01", "w10", "w11"):
            WCT[tag] = wpool.tile([128, GPOS], bf16, tag=tag, name=tag)
        WCL = [WCT["w00"], WCT["w01"], WCT["w10"], WCT["w11"]]

        for sg in range(1):
            s = slice(0, 128)
            # ADF = 1 + 2*(1-sigmoid), in place over PSGN
            vec.tensor_scalar(PSGN[s, :], PSGN[s, :], 2.0, 1.0, ALU.mult, ALU.add)
            ADF = PSGN
            vec.tensor_tensor(V[s, :], POFF[s, :], bslp[s, :], op=ALU.add)
            vec.scalar_tensor_tensor(V[s, :], ADF[s, :], pnb[s, :], V[s, :],
                                     op0=ALU.mult, op1=ALU.add)
            # floor(V) robust to convert rounding: g=int(V); F=g-(g>V)
            vec.tensor_copy(I32T[s, :], V[s, :])
            vec.tensor_copy(F[s, :], I32T[s, :])
            vec.tensor_tensor(FRAC[s, :], F[s, :], V[s, :], op=ALU.is_gt)
            vec.tensor_tensor(F[s, :], F[s, :], FRAC[s, :], op=ALU.subtract)
            vec.tensor_tensor(FRAC[s, :], V[s, :], F[s, :], op=ALU.subtract)
            vec.tensor_scalar(QRB1[s, :], F[s, :], 1.0, 0.0, ALU.add, ALU.max)
            vec.tensor_scalar(F[s, :], F[s, :], 0.0, Hp - 1.0, ALU.max, ALU.min)
            QLT = F
            vec.tensor_scalar(R1[s, :], QRB1[s, :], Hp + 0.0, None, ALU.min)
            vec.tensor_scalar(QRB1[s, :], QRB1[s, :], Hp - 1.0, None, ALU.min)
            QRB = QRB1
            act.activation(MASK[s, :], V[s, :], ACTF.Abs, bias=mbias[s, :],
                           scale=1.0)
            vec.tensor_scalar(MASK[s, :], MASK[s, :], 63.5, None, ALU.is_gt)
            vec.tensor_tensor(MASK[s, :], MASK[s, :], FRAC[s, :], op=ALU.mult)
            vec.tensor_tensor(V[s, :], V[s, :], MASK[s, :], op=ALU.subtract)
            vec.tensor_scalar(V[s, :], V[s, :], 0.0, Hp - 1.0, ALU.max, ALU.min)
            vec.scalar_tensor_tensor(GLT[s, :], QLT[s, :], 1.0, V[s, :],
                                     op0=ALU.add, op1=ALU.subtract)
            vec.scalar_tensor_tensor(GRB[s, :], V[s, :], 1.0, QRB[s, :],
                                     op0=ALU.add, op1=ALU.subtract)
            # modulation mm = m * ad_m (valid on x-rows g*32..+9)
            vec.tensor_scalar(ADM[s, :], ADF[s, :], 2.0, -4.0, ALU.mult, ALU.add)
            vec.tensor_tensor(ADM[s, :], PSGM[s, :], ADM[s, :], op=ALU.mult)
            vec.tensor_tensor(T1[s, :], ADM[s, :], GLT[s, :], op=ALU.mult)
            vec.tensor_tensor(T2[s, :], ADM[s, :], GRB[s, :], op=ALU.mult)
            for g in range(NG):
                b0 = g * 32
                snc.dma_start(out=GLTY[b0:b0 + 9, :], in_=GLT[b0 + 9:b0 + 18, :])
                snc.dma_start(out=GRBY[b0:b0 + 9, :], in_=GRB[b0 + 9:b0 + 18, :])
            for (srcw, gy, tag) in ((T1, GLTY, "w00"), (T1, GRBY, "w01"),
                                    (T2, GLTY, "w10"), (T2, GRBY, "w11")):
                vec.tensor_tensor(WCT[tag][s, :], srcw[s, :], gy[s, :],
                                  op=ALU.mult)

            for g in range(NG):
                b0 = g * 32
                # idxf col (bl*128 + p*8 + s) <- idx[n, pos=bl*128+s*16+p]
                idxf = kpool.tile([9, GPOS], f32, tag="idxf")
                for k in range(2):
                    pi = psB.tile([9, 512], f32, tag="pi")
                    pe.matmul(pi[:], lcb[b0:b0 + 18, :],
                              R1[b0:b0 + 18, k * 512:(k + 1) * 512],
                              start=True, stop=True, tile_position=(b0, 0))
                    act.activation(
                        AP(idxf.tensor, idxf[:].offset + k * 512,
                           [[GPOS, 9], [128, 4], [1, 8], [8, 16]]),
                        pi[:], ACTF.Copy, scale=1.0)
                # idxt[p*8+s, bl*9+n] = idx[n, pos=bl*128+s*16+p] (i16)
                idxt = kpool.tile([128, NBLK * 9], i16, tag="idxt")
                for bl in range(NBLK):
                    tp = psB.tile([128, 9], f32, tag="tp")
                    pe.transpose(tp[:], idxf[:, bl * 128:(bl + 1) * 128],
                                 idb[0:9, 0:9])
                    vec.tensor_copy(idxt[:, bl * 9:bl * 9 + 9], tp[:])

                for h in range(2):
                    idxm = gpool.tile([16, HTOK // 16], i16, tag="idxm")
                    for bl in range(4):
                        act.dma_start(
                            out=AP(idxm.tensor, idxm[:].offset + bl * 72,
                                   [[HTOK // 16, 16], [9, 8], [1, 9]]),
                            in_=AP(idxt.tensor,
                                   idxt[:].offset + (h * 4 + bl) * 9,
                                   [[NBLK * 9, 128], [1, 9]]))
                    idxw = gpool.tile([16, HTOK // 16], i16, tag="idxw")
                    vec.tensor_copy(
                        AP(idxw.tensor, idxw[:].offset,
                           [[HTOK // 16, 16], [32, 9], [1, 32]]),
                        AP(idxm.tensor, idxm[:].offset,
                           [[HTOK // 16, 16], [1, 9], [9, 32]]))
                    idxr = cpool.tile([128, HTOK // 16], i16,
                                      tag=f"idxr{g}{h}", name=f"idxr{g}{h}")
                    snc.dma_start(out=idxr[0:16, :], in_=idxw[:])
                    snc.dma_start(out=idxr[16:32, :], in_=idxw[:])
                    snc.dma_start(out=idxr[32:64, :], in_=idxr[0:32, :])
                    snc.dma_start(out=idxr[64:128, :], in_=idxr[0:64, :])
                    idxr_t[(g, h)] = idxr

        # ================= phase D: gather/apply/matmul pipeline ==========
        PIT = 2 * HTOK

        def build_wrb(k):
            g_, h_ = k // 2, k % 2
            b0_ = g_ * 32
            WRB = wrbpool.tile([128, 2 * HTOK], bf16, tag="wrb", name="wrb")
            weng = act if k % 2 else snc
            for r_ in range(2):
                for pp in range(2):
                    weng.dma_start(
                        out=AP(WRB.tensor,
                               WRB[:].offset + pp * PIT + r_ * 128,
                               [[PIT, 1], [1024, 9], [256, 4], [1, 128]]),
                        in_=WCL[r_ * 2 + pp][b0_:b0_ + 9,
                                             h_ * 512:(h_ + 1) * 512])
            for kk in (2, 4, 8, 16, 32, 64):
                weng.dma_start(
                    out=AP(WRB.tensor, WRB[:].offset + kk * PIT,
                           [[PIT, kk], [1, PIT]]),
                    in_=AP(WRB.tensor, WRB[:].offset,
                           [[PIT, kk], [1, PIT]]))
            return WRB

        WRB_t = {0: build_wrb(0), 1: build_wrb(1)}
        for g in range(NG):
            b0 = g * 32
            for h in range(2):
                idxr = idxr_t[(g, h)]
                # gather (non-transpose, 512B patch tokens, 2 queues)
                Gp = gpool.tile([128, 36 * 256], bf16, tag="Gp")
                for qh in range(2):
                    qn = (2 * (g * 2 + h) + qh) % 4
                    gp.dma_gather(
                        out_ap=AP(Gp.tensor, Gp[:].offset + qh * 18 * 256,
                                  [[36 * 256, 128], [256, 18], [1, 256]]),
                        in_ap=xe[:],
                        idxs_ap=idxr[:, qh * (QTOK // 16):(qh + 1) * (QTOK // 16)],
                        num_idxs=QTOK,
                        num_idxs_reg=QTOK,
                        elem_size=256,
                        elem_step=256,
                        transpose=False,
                        single_packet=False,
                        queue_num=qn)

                # xbar transpose to channel-major, split per queue-gather so
                # the first half overlaps the second gather:
                # G2[q, b*256 + r*128 + p] = Gp[p, b*256 + r*128 + q]
                G2 = g2pool.tile([128, 2 * HTOK], bf16, tag="G2")
                teng = snc if (g * 2 + h) % 2 else act
                for qh in range(2):
                    teng.dma_start(
                        out=AP(G2.tensor, G2[:].offset + qh * HTOK,
                               [[2 * HTOK, 128], [128, 36], [1, 128]]),
                        in_=Gp[:, qh * 18 * 256:(qh + 1) * 18 * 256],
                        transpose=True)

                WRB = WRB_t.pop(g * 2 + h)

                # apply weights (in place, contiguous)
                for hh in range(2):
                    slh = slice(hh * HTOK, (hh + 1) * HTOK)
                    any_.tensor_tensor(G2[:, slh], G2[:, slh], WRB[:, slh],
                                       op=ALU.mult)

                # final matmuls
                po = psA.tile([64, 512], f32, tag="po")
                for tt in range(18):
                    r_, n_ = tt // 9, tt % 9
                    rhs = AP(G2.tensor,
                             G2[:].offset + n_ * 1024 + r_ * 128,
                             [[2 * HTOK, 128], [256, 4], [1, 128]])
                    pe.matmul(po[:], w3b[:, tt * 64:(tt + 1) * 64], rhs,
                              start=(tt == 0), stop=(tt == 17))
                oc = wpool.tile([64, 512], f32, tag="oc")
                vec.tensor_copy(oc[:], po[:])
                snc.dma_start(
                    out=t['outp'][:, g * GPOS + h * 512:g * GPOS + (h + 1) * 512],
                    in_=oc[:])
                if g * 2 + h + 2 < 2 * NG:
                    WRB_t[g * 2 + h + 2] = build_wrb(g * 2 + h + 2)


def _build():
    import concourse.bacc as bacc
    import concourse.tile as tile
    import concourse.mybir as mybir
    dt = mybir.dt

    nc = bacc.Bacc("TRN2", target_bir_lowering=False, debug=False,
                   num_swdge_queues=4)
    t = {}
    specs = [
        ('xs', [C, HSH + 2, W], dt.float32),
        ('xe', [NPATCH, 256], dt.bfloat16),
        ('wt', [128, 6 * M_CONV], dt.float32),
        ('bp128', [128, 1], dt.float32),
        ('pn128', [128, 1], dt.float32),
        ('lconst', [128, 9], dt.float32),
        ('w3', [128, 18 * 64], dt.bfloat16),
        ('ident', [128, 128], dt.float32),
        ('basep', [128, GPOS], dt.float32),
    ]
    for name, shape, d in specs:
        t[name] = nc.dram_tensor(name, shape, d, kind="ExternalInput").ap()
    t['outp'] = nc.dram_tensor('outp', [64, NPOS], dt.float32,
                               kind="ExternalOutput").ap()
    with tile.TileContext(nc) as tc:
        _emit(nc, tc, t)
    nc.compile()
    return nc


def kernel(x, w_p, b_p, w_m, w_ad, w_conv):
    from concourse.bass_utils import run_bass_kernel_spmd

    x = np.asarray(x, np.float32)
    consts = _prep_consts(np.asarray(w_p, np.float32), np.asarray(b_p, np.float32),
                          np.asarray(w_m, np.float32), np.asarray(w_ad, np.float32),
                          np.asarray(w_conv, np.float32))
    for b in range(B):
        _cache[('xe', b)] = _prep_table(x[b])
    if 'nc' not in _cache:
        _cache['nc'] = _build()
    nc = _cache['nc']

    in_maps = [_prep_core_inputs(c, x, consts) for c in range(NCORES)]
    res = run_bass_kernel_spmd(nc, in_maps, list(range(NCORES)))
    _cache['last_results'] = res

    out = np.zeros((B, 64, H, W), np.float32)
    for c in range(NCORES):
        b, hc = c // 4, c % 4
        out[b, :, hc * HSH:(hc + 1) * HSH, :] = \
            res.results[c]['outp'].reshape(64, HSH, W)
    return out
